# revision 8
# baseline (speedup 1.0000x reference)
"""Trainium2 Bass kernel for CrossLayerFeatureUpdate (2x2 block-pooled GNN
message passing between a 256x256 grid and its 128x128 downsample).

Math notes (from the reference adjacency construction):
  - orig=256, down=128, f=orig//down=2, so the 4 neighbors of down node
    (di,dj) are exactly the non-overlapping 2x2 block {2di,2di+1}x{2dj,2dj+1}.
    Every orig node has degree exactly 1 => all adjacency vals == 0.5.
  - down path:  H_new = relu(LN((0.5 * pool2x2sum(H_orig)) @ W_o2n.T))
  - up path:    rows of (0.5 * upsample(H_down)) @ W_n2o.T are identical for
    the 4 children of one down node, and LN/relu act rowwise, so we compute
    y = relu(LN((0.5 * H_down) @ W_n2o.T)) once per down node and write it
    to the 4 child positions.
  - The 0.5 is folded into the weight matrices on the host.

Layout trick: one orig grid-row r (256 cols x 256 feats, contiguous) viewed
as [128, 512] puts col 2p features in free [0:256] and col 2p+1 features in
free [256:512] of partition p. So 2x2 pooling becomes plain aligned adds of
[128,256] slices of two contiguous row loads, and the up-path duplication is
one [128,512] tile = [y | y] written to two consecutive rows.

Sharding: 8 cores = batch(4) x row-half(2). Per core: 32MB+8MB in,
32MB+8MB out, all contiguous DMA. No cross-core communication.
"""

import os
import numpy as np

BATCH, ORIG, DOWN, F = 4, 256, 128, 256
EPS = 1e-5
NCORES = 8
RPC = ORIG // 2   # orig grid-rows per core (128)
DRC = DOWN // 2   # down grid-rows per core (64)

LAST_EXEC_NS = None
LAST_RESULTS = None

_compiled_cache = {}


def _make_tile_context_cls(tile_mod):
    import concourse.bass  # noqa: F401
    from concourse import mybir
    from concourse.vector_clock import ScopedClock

    # opcodes executed by DMA queues (waits ride in DGE descriptors, which
    # have no small per-instruction wait-slot limit)
    _DMA_INSTS = tuple(
        getattr(mybir, n)
        for n in ("InstDMACopy", "InstDMATranspose", "InstTensorLoad",
                  "InstTensorSave")
        if hasattr(mybir, n)
    )

    class TileContextSplitDrain(tile_mod.TileContext):
        """TileContext patched for this walrus build, which rejects >1-2
        sync-wait commands on a single engine instruction: excess waits are
        hoisted onto single-wait EventSemaphore instructions emitted just
        before the instruction on the same engine."""

        MAX_INST_WAITS = 1
        _dma_helper_sem = None
        _dma_helper_count = 0

        def _emit_wait_es(self, engine, wait, update=None):
            es = mybir.InstEventSemaphore(
                name=self.nc.get_next_instruction_name(), ins=[], outs=[])
            es.engine = engine
            es.sync_info = mybir.SyncInfo(
                on_wait=[wait] if wait is not None else [],
                on_update=[update] if update is not None else [])
            super()._add_instruction(es)

        def _add_instruction(self, inst):
            si = inst.sync_info
            if (
                si is not None
                and si.on_wait
                and len(si.on_wait) > self.MAX_INST_WAITS
                and inst.engine != mybir.EngineType.Unassigned
            ):
                waits = list(si.on_wait)
                if isinstance(inst, _DMA_INSTS):
                    # DMA descriptors execute on DGE queues, not the issuing
                    # engine stream, so an in-stream EventSemaphore alone
                    # would not gate them. Hoist every wait onto the issuing
                    # engine, then have the last hoisted wait bump a helper
                    # semaphore that becomes the DMA's single wait.
                    if self._dma_helper_sem is None:
                        type(self)._dma_helper_sem = self.nc.alloc_semaphore(
                            "dma_wait_helper")
                    for w in waits[:-1]:
                        self._emit_wait_es(inst.engine, w)
                    type(self)._dma_helper_count += 1
                    cnt = self._dma_helper_count
                    upd = mybir.SyncUpdate(
                        ant_name=self._dma_helper_sem.name,
                        id=self._dma_helper_sem.num,
                        sync_type="semaphore",
                        update_mode="sem-inc", update_value=1)
                    self._emit_wait_es(inst.engine, waits[-1], upd)
                    helper_wait = mybir.SyncWait(
                        ant_name=self._dma_helper_sem.name,
                        id=self._dma_helper_sem.num,
                        sync_type="semaphore",
                        wait_mode="sem-ge-imm", wait_value=cnt)
                    inst.sync_info = mybir.SyncInfo(
                        on_wait=[helper_wait],
                        on_update=list(si.on_update or []))
                else:
                    keep = waits[-self.MAX_INST_WAITS:]
                    for w in waits[:-self.MAX_INST_WAITS]:
                        self._emit_wait_es(inst.engine, w)
                    inst.sync_info = mybir.SyncInfo(
                        on_wait=keep, on_update=list(si.on_update or []))
            super()._add_instruction(inst)

        def _drain_and_barrier(self, tick_clock, wait_clock):
            nc = self.nc
            scratch = nc.sync.nop()
            wait_clock.add_sem_waits(
                scratch.ins, ScopedClock({None: tick_clock.global_clock})
            )
            si = scratch.ins.sync_info
            waits = list(si.on_wait) if (si and si.on_wait) else []
            scratch.ins.sync_info = None
            assert self.sems is not None
            num2sem = {h.num: h for h in self.sems.allocated().values()}
            for w in waits:
                nc.sync.wait_ge(num2sem[w.id], w.wait_value)
            nc.sync.drain()
            nc.all_engine_barrier()
            popped = nc._tile_sem_poison_stack.pop()
            assert popped is self._sem_poison
            sems = list(self.sems.allocated().values())
            if self._dma_helper_sem is not None:
                sems.append(self._dma_helper_sem)
            nc.clear_and_free_semaphores(sems)
            nc.all_engine_barrier()

    return TileContextSplitDrain


def _build_program(apply_gb):
    import concourse.bass as bass
    import concourse.tile as tile
    from concourse import mybir

    f32 = mybir.dt.float32
    f32r = mybir.dt.float32r
    ACT = mybir.ActivationFunctionType
    ALU = mybir.AluOpType

    nc = bass.Bass("TRN2", target_bir_lowering=False, debug=False,
                   num_devices=NCORES)

    ho_in = nc.dram_tensor("ho", [RPC, 128, 512], f32, kind="ExternalInput")
    hd_in = nc.dram_tensor("hd", [DRC, 128, 256], f32, kind="ExternalInput")
    wo_in = nc.dram_tensor("wo", [2, 128, 256], f32, kind="ExternalInput")
    wn_in = nc.dram_tensor("wn", [2, 128, 256], f32, kind="ExternalInput")
    id_in = nc.dram_tensor("ident", [128, 128], f32, kind="ExternalInput")
    if apply_gb:
        gb_in = {
            n: nc.dram_tensor(n, [128, 256], f32, kind="ExternalInput")
            for n in ("go", "bo", "gn", "bn")
        }
    hn_out = nc.dram_tensor("hn", [DRC, 128, 256], f32, kind="ExternalOutput")
    hu_out = nc.dram_tensor("hu", [RPC, 128, 512], f32, kind="ExternalOutput")

    TC = _make_tile_context_cls(tile)

    with TC(nc) as tc:
        import contextlib
        with contextlib.ExitStack() as ctx:
            consts = ctx.enter_context(tc.tile_pool(name="consts", bufs=1))
            rin = ctx.enter_context(tc.tile_pool(name="rin", bufs=3))
            hdin = ctx.enter_context(tc.tile_pool(name="hdin", bufs=3))
            dpool = ctx.enter_context(tc.tile_pool(name="dmt", bufs=3))
            outp = ctx.enter_context(tc.tile_pool(name="outp", bufs=3))
            stats = ctx.enter_context(tc.tile_pool(name="stats", bufs=8))
            tpool = ctx.enter_context(
                tc.tile_pool(name="tp", bufs=2, space="PSUM"))
            opool = ctx.enter_context(
                tc.tile_pool(name="op", bufs=2, space="PSUM"))

            w_o = consts.tile([128, 2, 256], f32r)
            w_n = consts.tile([128, 2, 256], f32r)
            ident = consts.tile([128, 128], f32)
            eps_t = consts.tile([128, 1], f32)
            nc.vector.memset(eps_t[:], EPS)
            for c in (0, 1):
                nc.gpsimd.dma_start(out=w_o[:, c, :], in_=wo_in[c])
                nc.gpsimd.dma_start(out=w_n[:, c, :], in_=wn_in[c])
            nc.sync.dma_start(out=ident[:], in_=id_in[:])
            if apply_gb:
                gb_sb = {}
                for n in ("go", "bo", "gn", "bn"):
                    t = consts.tile([128, 256], f32, tag=f"gb_{n}")
                    nc.sync.dma_start(out=t[:], in_=gb_in[n][:])
                    gb_sb[n] = t

            def layernorm_relu(psum_t, out_ap, g_tile, b_tile):
                """out = relu(LN(psum_t) * g + b); writes [128,256] to out_ap."""
                st = stats.tile([128, 6], f32, tag="st")
                nc.vector.bn_stats(out=st[:], in_=psum_t[:])
                mv = stats.tile([128, 2], f32, tag="mv")
                nc.vector.bn_aggr(out=mv[:], in_=st[:])
                sd = stats.tile([128, 1], f32, tag="sd")
                nc.scalar.activation(out=sd[:], in_=mv[:, 1:2], func=ACT.Sqrt,
                                     bias=eps_t[:], scale=1.0)
                rstd = stats.tile([128, 1], f32, tag="rstd")
                nc.vector.reciprocal(out=rstd[:], in_=sd[:])
                nmr = stats.tile([128, 1], f32, tag="nmr")
                nc.vector.scalar_tensor_tensor(
                    out=nmr[:], in0=mv[:, 0:1], scalar=-1.0, in1=rstd[:],
                    op0=ALU.mult, op1=ALU.mult)
                if g_tile is None:
                    nc.scalar.activation(out=out_ap, in_=psum_t[:],
                                         func=ACT.Relu, scale=rstd[:],
                                         bias=nmr[:])
                else:
                    xn = stats.tile([128, 256], f32, tag="xn")
                    nc.scalar.activation(out=xn[:], in_=psum_t[:],
                                         func=ACT.Identity, scale=rstd[:],
                                         bias=nmr[:])
                    nc.vector.tensor_mul(out=xn[:], in0=xn[:], in1=g_tile[:])
                    nc.vector.tensor_add(out=xn[:], in0=xn[:], in1=b_tile[:])
                    nc.vector.tensor_scalar_max(out=out_ap, in0=xn[:],
                                                scalar1=0.0)

            for di in range(DRC):
                # ---------- down path: pool 2x2 -> matmul -> LN -> relu ----
                rp = rin.tile([128, 1024], f32, tag="rp")
                nc.sync.dma_start(out=rp[:, 0:512], in_=ho_in[2 * di])
                nc.sync.dma_start(out=rp[:, 512:1024], in_=ho_in[2 * di + 1])

                tps = tpool.tile([128, 256], f32, tag="tp")
                for c in (0, 1):
                    for gi, base in enumerate((0, 256, 512, 768)):
                        nc.tensor.matmul(
                            out=tps[:, c * 128:(c + 1) * 128],
                            lhsT=rp[:, base + c * 128: base + c * 128 + 128],
                            rhs=ident[:],
                            is_transpose=True,
                            start=(gi == 0), stop=(gi == 3),
                        )
                dmT = dpool.tile([128, 256], f32r, tag="dmT")
                nc.scalar.copy(out=dmT[:], in_=tps[:])

                ops = opool.tile([128, 256], f32, tag="op")
                for c in (0, 1):
                    nc.tensor.matmul(
                        out=ops[:],
                        lhsT=dmT[:, c * 128:(c + 1) * 128],
                        rhs=w_o[:, c, :],
                        start=(c == 0), stop=(c == 1),
                    )

                hnt = outp.tile([128, 256], f32, tag="hnt")
                layernorm_relu(ops, hnt[:],
                               gb_sb["go"] if apply_gb else None,
                               gb_sb["bo"] if apply_gb else None)
                nc.sync.dma_start(out=hn_out[di], in_=hnt[:])

                # ---------- up path: matmul -> LN -> relu -> duplicate -----
                hdt = hdin.tile([128, 256], f32, tag="hdt")
                nc.sync.dma_start(out=hdt[:], in_=hd_in[di])

                tps2 = tpool.tile([128, 256], f32, tag="tp")
                for c in (0, 1):
                    nc.tensor.matmul(
                        out=tps2[:, c * 128:(c + 1) * 128],
                        lhsT=hdt[:, c * 128:(c + 1) * 128],
                        rhs=ident[:],
                        is_transpose=True,
                        start=True, stop=True,
                    )
                dmT2 = dpool.tile([128, 256], f32r, tag="dmT")
                nc.scalar.copy(out=dmT2[:], in_=tps2[:])

                ops2 = opool.tile([128, 256], f32, tag="op")
                for c in (0, 1):
                    nc.tensor.matmul(
                        out=ops2[:],
                        lhsT=dmT2[:, c * 128:(c + 1) * 128],
                        rhs=w_n[:, c, :],
                        start=(c == 0), stop=(c == 1),
                    )

                y2 = outp.tile([128, 512], f32, tag="y2")
                layernorm_relu(ops2, y2[:, 0:256],
                               gb_sb["gn"] if apply_gb else None,
                               gb_sb["bn"] if apply_gb else None)
                nc.vector.tensor_copy(out=y2[:, 256:512], in_=y2[:, 0:256])
                nc.sync.dma_start(out=hu_out[2 * di], in_=y2[:])
                nc.sync.dma_start(out=hu_out[2 * di + 1], in_=y2[:])

    return nc


def _get_program(apply_gb):
    key = bool(apply_gb)
    if key not in _compiled_cache:
        _compiled_cache[key] = _build_program(key)
    return _compiled_cache[key]


def _install_trace_shim():
    """Register the NTFF profile hook so trace=True works (timing)."""
    import sys, types
    if "antenv.axon_hooks" in sys.modules:
        return
    try:
        from trn_agent_boot.trn_boot import _ntff_profile_via_ctypes
        hook = _ntff_profile_via_ctypes('/opt/axon/libaxon_pjrt.so')
    except Exception:
        return
    mod = types.ModuleType("antenv.axon_hooks")
    mod.get_axon_ntff_profile_hook = lambda: hook
    mod.set_axon_ntff_profile_hook = lambda h: None
    sys.modules["antenv.axon_hooks"] = mod
    import concourse.bass_utils as bu
    bu.upload_artifacts = lambda tmpdir: "local://" + str(tmpdir)


def kernel(H_orig, H_down, W_o2n, W_n2o, g_o2n, b_o2n, g_n2o, b_n2o):
    global LAST_EXEC_NS, LAST_RESULTS
    from concourse.bass_utils import run_bass_kernel_spmd

    H_orig = np.ascontiguousarray(np.asarray(H_orig), dtype=np.float32)
    H_down = np.ascontiguousarray(np.asarray(H_down), dtype=np.float32)
    W_o2n = np.asarray(W_o2n, dtype=np.float32)
    W_n2o = np.asarray(W_n2o, dtype=np.float32)
    g_o2n = np.asarray(g_o2n, dtype=np.float32)
    b_o2n = np.asarray(b_o2n, dtype=np.float32)
    g_n2o = np.asarray(g_n2o, dtype=np.float32)
    b_n2o = np.asarray(b_n2o, dtype=np.float32)

    apply_gb = not (
        np.all(g_o2n == 1.0) and np.all(b_o2n == 0.0)
        and np.all(g_n2o == 1.0) and np.all(b_n2o == 0.0)
    )

    trace = bool(int(os.environ.get("KERNEL_TRACE", "0")))
    if trace:
        _install_trace_shim()

    nc = _get_program(apply_gb)

    ho_v = H_orig.reshape(BATCH, ORIG, 128, 512)
    hd_v = H_down.reshape(BATCH, DOWN, 128, 256)
    wo = np.ascontiguousarray((0.5 * W_o2n.T).reshape(2, 128, 256))
    wn = np.ascontiguousarray((0.5 * W_n2o.T).reshape(2, 128, 256))
    ident = np.eye(128, dtype=np.float32)

    in_maps = []
    for core in range(NCORES):
        b, h = divmod(core, 2)
        im = {
            "ho": ho_v[b, RPC * h: RPC * (h + 1)],
            "hd": hd_v[b, DRC * h: DRC * (h + 1)],
            "wo": wo, "wn": wn, "ident": ident,
        }
        if apply_gb:
            im["go"] = np.ascontiguousarray(
                np.broadcast_to(g_o2n, (128, 256)))
            im["bo"] = np.ascontiguousarray(
                np.broadcast_to(b_o2n, (128, 256)))
            im["gn"] = np.ascontiguousarray(
                np.broadcast_to(g_n2o, (128, 256)))
            im["bn"] = np.ascontiguousarray(
                np.broadcast_to(b_n2o, (128, 256)))
        in_maps.append(im)

    res = run_bass_kernel_spmd(nc, in_maps, core_ids=list(range(NCORES)),
                               trace=trace)
    LAST_EXEC_NS = res.exec_time_ns
    LAST_RESULTS = res

    H_up = np.empty((BATCH, ORIG * ORIG, F), dtype=np.float32)
    H_new = np.empty((BATCH, DOWN * DOWN, F), dtype=np.float32)
    hu_v = H_up.reshape(BATCH, 2, RPC, 128, 512)
    hn_v = H_new.reshape(BATCH, 2, DRC, 128, 256)
    for core in range(NCORES):
        b, h = divmod(core, 2)
        hu_v[b, h] = res.results[core]["hu"]
        hn_v[b, h] = res.results[core]["hn"]
    return H_up, H_new


# revision 9
# speedup vs baseline: 1.2046x; 1.2046x over previous
"""Trainium2 Bass kernel for CrossLayerFeatureUpdate (2x2 block-pooled GNN
message passing between a 256x256 grid and its 128x128 downsample).

Math notes (from the reference adjacency construction):
  - orig=256, down=128, f=orig//down=2, so the 4 neighbors of down node
    (di,dj) are exactly the non-overlapping 2x2 block {2di,2di+1}x{2dj,2dj+1}.
    Every orig node has degree exactly 1 => all adjacency vals == 0.5.
  - down path:  H_new = relu(LN((0.5 * pool2x2sum(H_orig)) @ W_o2n.T))
  - up path:    rows of (0.5 * upsample(H_down)) @ W_n2o.T are identical for
    the 4 children of one down node, and LN/relu act rowwise, so we compute
    y = relu(LN((0.5 * H_down) @ W_n2o.T)) once per down node and write it
    to the 4 child positions.
  - The 0.5 is folded into the weight matrices on the host.

Layout trick: one orig grid-row r (256 cols x 256 feats, contiguous) viewed
as [128, 512] puts col 2p features in free [0:256] and col 2p+1 features in
free [256:512] of partition p. So 2x2 pooling becomes plain aligned adds of
[128,256] slices of two contiguous row loads, and the up-path duplication is
one [128,512] tile = [y | y] written to two consecutive rows.

Sharding: 8 cores = batch(4) x row-half(2). Per core: 32MB+8MB in,
32MB+8MB out, all contiguous DMA. No cross-core communication.
"""

import os
import numpy as np

BATCH, ORIG, DOWN, F = 4, 256, 128, 256
EPS = 1e-5
NCORES = 8
RPC = ORIG // 2   # orig grid-rows per core (128)
DRC = DOWN // 2   # down grid-rows per core (64)

LAST_EXEC_NS = None
LAST_RESULTS = None

_compiled_cache = {}


def _make_tile_context_cls(tile_mod):
    import concourse.bass  # noqa: F401
    from concourse import mybir
    from concourse.vector_clock import ScopedClock

    # opcodes executed by DMA queues (waits ride in DGE descriptors, which
    # have no small per-instruction wait-slot limit)
    _DMA_INSTS = tuple(
        getattr(mybir, n)
        for n in ("InstDMACopy", "InstDMATranspose", "InstTensorLoad",
                  "InstTensorSave")
        if hasattr(mybir, n)
    )

    class TileContextSplitDrain(tile_mod.TileContext):
        """TileContext patched for this walrus build, which rejects >1-2
        sync-wait commands on a single engine instruction: excess waits are
        hoisted onto single-wait EventSemaphore instructions emitted just
        before the instruction on the same engine."""

        MAX_INST_WAITS = 1
        _dma_helper_sem = None
        _dma_helper_count = 0

        def _emit_wait_es(self, engine, wait, update=None):
            es = mybir.InstEventSemaphore(
                name=self.nc.get_next_instruction_name(), ins=[], outs=[])
            es.engine = engine
            es.sync_info = mybir.SyncInfo(
                on_wait=[wait] if wait is not None else [],
                on_update=[update] if update is not None else [])
            super()._add_instruction(es)

        def _add_instruction(self, inst):
            si = inst.sync_info
            if (
                si is not None
                and si.on_wait
                and len(si.on_wait) > self.MAX_INST_WAITS
                and inst.engine != mybir.EngineType.Unassigned
            ):
                waits = list(si.on_wait)
                if isinstance(inst, _DMA_INSTS):
                    # DMA descriptors execute on DGE queues, not the issuing
                    # engine stream, so an in-stream EventSemaphore alone
                    # would not gate them. Hoist every wait onto the issuing
                    # engine, then have the last hoisted wait bump a helper
                    # semaphore that becomes the DMA's single wait.
                    if self._dma_helper_sem is None:
                        type(self)._dma_helper_sem = self.nc.alloc_semaphore(
                            "dma_wait_helper")
                    for w in waits[:-1]:
                        self._emit_wait_es(inst.engine, w)
                    type(self)._dma_helper_count += 1
                    cnt = self._dma_helper_count
                    upd = mybir.SyncUpdate(
                        ant_name=self._dma_helper_sem.name,
                        id=self._dma_helper_sem.num,
                        sync_type="semaphore",
                        update_mode="sem-inc", update_value=1)
                    self._emit_wait_es(inst.engine, waits[-1], upd)
                    helper_wait = mybir.SyncWait(
                        ant_name=self._dma_helper_sem.name,
                        id=self._dma_helper_sem.num,
                        sync_type="semaphore",
                        wait_mode="sem-ge-imm", wait_value=cnt)
                    inst.sync_info = mybir.SyncInfo(
                        on_wait=[helper_wait],
                        on_update=list(si.on_update or []))
                else:
                    keep = waits[-self.MAX_INST_WAITS:]
                    for w in waits[:-self.MAX_INST_WAITS]:
                        self._emit_wait_es(inst.engine, w)
                    inst.sync_info = mybir.SyncInfo(
                        on_wait=keep, on_update=list(si.on_update or []))
            super()._add_instruction(inst)

        def _drain_and_barrier(self, tick_clock, wait_clock):
            nc = self.nc
            scratch = nc.sync.nop()
            wait_clock.add_sem_waits(
                scratch.ins, ScopedClock({None: tick_clock.global_clock})
            )
            si = scratch.ins.sync_info
            waits = list(si.on_wait) if (si and si.on_wait) else []
            scratch.ins.sync_info = None
            assert self.sems is not None
            num2sem = {h.num: h for h in self.sems.allocated().values()}
            for w in waits:
                nc.sync.wait_ge(num2sem[w.id], w.wait_value)
            nc.sync.drain()
            nc.all_engine_barrier()
            popped = nc._tile_sem_poison_stack.pop()
            assert popped is self._sem_poison
            sems = list(self.sems.allocated().values())
            if self._dma_helper_sem is not None:
                sems.append(self._dma_helper_sem)
            nc.clear_and_free_semaphores(sems)
            nc.all_engine_barrier()

    return TileContextSplitDrain


def _build_program(apply_gb):
    import concourse.bass as bass
    import concourse.tile as tile
    from concourse import mybir

    f32 = mybir.dt.float32
    f32r = mybir.dt.float32r
    ACT = mybir.ActivationFunctionType
    ALU = mybir.AluOpType

    nc = bass.Bass("TRN2", target_bir_lowering=False, debug=False,
                   num_devices=NCORES)

    ho_in = nc.dram_tensor("ho", [RPC, 128, 512], f32, kind="ExternalInput")
    hd_in = nc.dram_tensor("hd", [DRC, 128, 256], f32, kind="ExternalInput")
    wo_in = nc.dram_tensor("wo", [2, 128, 256], f32, kind="ExternalInput")
    wn_in = nc.dram_tensor("wn", [2, 128, 256], f32, kind="ExternalInput")
    id_in = nc.dram_tensor("ident", [128, 128], f32, kind="ExternalInput")
    if apply_gb:
        gb_in = {
            n: nc.dram_tensor(n, [128, 256], f32, kind="ExternalInput")
            for n in ("go", "bo", "gn", "bn")
        }
    hn_out = nc.dram_tensor("hn", [DRC, 128, 256], f32, kind="ExternalOutput")
    hu_out = nc.dram_tensor("hu", [RPC, 128, 512], f32, kind="ExternalOutput")

    TC = _make_tile_context_cls(tile)

    with TC(nc) as tc:
        import contextlib
        with contextlib.ExitStack() as ctx:
            consts = ctx.enter_context(tc.tile_pool(name="consts", bufs=1))
            rin = ctx.enter_context(tc.tile_pool(name="rin", bufs=4))
            hdin = ctx.enter_context(tc.tile_pool(name="hdin", bufs=4))
            dpool = ctx.enter_context(tc.tile_pool(name="dmt", bufs=4))
            outp = ctx.enter_context(tc.tile_pool(name="outp", bufs=4))
            stats = ctx.enter_context(tc.tile_pool(name="stats", bufs=12))
            tpool = ctx.enter_context(
                tc.tile_pool(name="tp", bufs=3, space="PSUM"))
            opool = ctx.enter_context(
                tc.tile_pool(name="op", bufs=3, space="PSUM"))

            w_o = consts.tile([128, 2, 256], f32r)
            w_n = consts.tile([128, 2, 256], f32r)
            ident = consts.tile([128, 128], f32)
            eps_t = consts.tile([128, 1], f32)
            nc.vector.memset(eps_t[:], EPS)
            for c in (0, 1):
                nc.gpsimd.dma_start(out=w_o[:, c, :], in_=wo_in[c])
                nc.gpsimd.dma_start(out=w_n[:, c, :], in_=wn_in[c])
            nc.sync.dma_start(out=ident[:], in_=id_in[:])
            if apply_gb:
                gb_sb = {}
                for n in ("go", "bo", "gn", "bn"):
                    t = consts.tile([128, 256], f32, tag=f"gb_{n}")
                    nc.sync.dma_start(out=t[:], in_=gb_in[n][:])
                    gb_sb[n] = t

            def layernorm_relu(psum_t, out_ap, g_tile, b_tile):
                """out = relu(LN(psum_t) * g + b); writes [128,256] to out_ap."""
                st = stats.tile([128, 6], f32, tag="st")
                nc.vector.bn_stats(out=st[:], in_=psum_t[:])
                mv = stats.tile([128, 2], f32, tag="mv")
                nc.vector.bn_aggr(out=mv[:], in_=st[:])
                sd = stats.tile([128, 1], f32, tag="sd")
                nc.scalar.activation(out=sd[:], in_=mv[:, 1:2], func=ACT.Sqrt,
                                     bias=eps_t[:], scale=1.0)
                rstd = stats.tile([128, 1], f32, tag="rstd")
                nc.vector.reciprocal(out=rstd[:], in_=sd[:])
                nmr = stats.tile([128, 1], f32, tag="nmr")
                nc.vector.scalar_tensor_tensor(
                    out=nmr[:], in0=mv[:, 0:1], scalar=-1.0, in1=rstd[:],
                    op0=ALU.mult, op1=ALU.mult)
                if g_tile is None:
                    nc.scalar.activation(out=out_ap, in_=psum_t[:],
                                         func=ACT.Relu, scale=rstd[:],
                                         bias=nmr[:])
                else:
                    xn = stats.tile([128, 256], f32, tag="xn")
                    nc.scalar.activation(out=xn[:], in_=psum_t[:],
                                         func=ACT.Identity, scale=rstd[:],
                                         bias=nmr[:])
                    nc.vector.tensor_mul(out=xn[:], in0=xn[:], in1=g_tile[:])
                    nc.vector.tensor_add(out=xn[:], in0=xn[:], in1=b_tile[:])
                    nc.vector.tensor_scalar_max(out=out_ap, in0=xn[:],
                                                scalar1=0.0)

            for di in range(DRC):
                # ---------- down path: pool 2x2 -> matmul -> LN -> relu ----
                rp = rin.tile([128, 1024], f32, tag="rp")
                nc.sync.dma_start(
                    out=rp[:].rearrange("p (r q) -> p r q", r=2),
                    in_=ho_in[2 * di: 2 * di + 2].rearrange("r p q -> p r q"))

                tps = tpool.tile([128, 256], f32, tag="tp")
                for c in (0, 1):
                    for gi, base in enumerate((0, 256, 512, 768)):
                        nc.tensor.matmul(
                            out=tps[:, c * 128:(c + 1) * 128],
                            lhsT=rp[:, base + c * 128: base + c * 128 + 128],
                            rhs=ident[:],
                            is_transpose=True,
                            start=(gi == 0), stop=(gi == 3),
                        )
                dmT = dpool.tile([128, 256], f32r, tag="dmT")
                nc.scalar.copy(out=dmT[:], in_=tps[:])

                ops = opool.tile([128, 256], f32, tag="op")
                for c in (0, 1):
                    nc.tensor.matmul(
                        out=ops[:],
                        lhsT=dmT[:, c * 128:(c + 1) * 128],
                        rhs=w_o[:, c, :],
                        start=(c == 0), stop=(c == 1),
                    )

                hnt = outp.tile([128, 256], f32, tag="hnt")
                layernorm_relu(ops, hnt[:],
                               gb_sb["go"] if apply_gb else None,
                               gb_sb["bo"] if apply_gb else None)
                nc.gpsimd.dma_start(out=hn_out[di], in_=hnt[:])

                # ---------- up path: matmul -> LN -> relu -> duplicate -----
                hdt = hdin.tile([128, 256], f32, tag="hdt")
                nc.gpsimd.dma_start(out=hdt[:], in_=hd_in[di])

                tps2 = tpool.tile([128, 256], f32, tag="tp")
                for c in (0, 1):
                    nc.tensor.matmul(
                        out=tps2[:, c * 128:(c + 1) * 128],
                        lhsT=hdt[:, c * 128:(c + 1) * 128],
                        rhs=ident[:],
                        is_transpose=True,
                        start=True, stop=True,
                    )
                dmT2 = dpool.tile([128, 256], f32r, tag="dmT")
                nc.scalar.copy(out=dmT2[:], in_=tps2[:])

                ops2 = opool.tile([128, 256], f32, tag="op")
                for c in (0, 1):
                    nc.tensor.matmul(
                        out=ops2[:],
                        lhsT=dmT2[:, c * 128:(c + 1) * 128],
                        rhs=w_n[:, c, :],
                        start=(c == 0), stop=(c == 1),
                    )

                y2 = outp.tile([128, 512], f32, tag="y2")
                layernorm_relu(ops2, y2[:, 0:256],
                               gb_sb["gn"] if apply_gb else None,
                               gb_sb["bn"] if apply_gb else None)
                nc.vector.tensor_copy(out=y2[:, 256:512], in_=y2[:, 0:256])
                nc.gpsimd.dma_start(out=hu_out[2 * di], in_=y2[:])
                nc.sync.dma_start(out=hu_out[2 * di + 1], in_=y2[:])

    return nc


def _get_program(apply_gb):
    key = bool(apply_gb)
    if key not in _compiled_cache:
        _compiled_cache[key] = _build_program(key)
    return _compiled_cache[key]


def _install_trace_shim():
    """Register the NTFF profile hook so trace=True works (timing)."""
    import sys, types
    if "antenv.axon_hooks" in sys.modules:
        return
    try:
        from trn_agent_boot.trn_boot import _ntff_profile_via_ctypes
        hook = _ntff_profile_via_ctypes('/opt/axon/libaxon_pjrt.so')
    except Exception:
        return
    mod = types.ModuleType("antenv.axon_hooks")
    mod.get_axon_ntff_profile_hook = lambda: hook
    mod.set_axon_ntff_profile_hook = lambda h: None
    sys.modules["antenv.axon_hooks"] = mod
    import concourse.bass_utils as bu
    bu.upload_artifacts = lambda tmpdir: "local://" + str(tmpdir)


def kernel(H_orig, H_down, W_o2n, W_n2o, g_o2n, b_o2n, g_n2o, b_n2o):
    global LAST_EXEC_NS, LAST_RESULTS
    from concourse.bass_utils import run_bass_kernel_spmd

    H_orig = np.ascontiguousarray(np.asarray(H_orig), dtype=np.float32)
    H_down = np.ascontiguousarray(np.asarray(H_down), dtype=np.float32)
    W_o2n = np.asarray(W_o2n, dtype=np.float32)
    W_n2o = np.asarray(W_n2o, dtype=np.float32)
    g_o2n = np.asarray(g_o2n, dtype=np.float32)
    b_o2n = np.asarray(b_o2n, dtype=np.float32)
    g_n2o = np.asarray(g_n2o, dtype=np.float32)
    b_n2o = np.asarray(b_n2o, dtype=np.float32)

    apply_gb = not (
        np.all(g_o2n == 1.0) and np.all(b_o2n == 0.0)
        and np.all(g_n2o == 1.0) and np.all(b_n2o == 0.0)
    )

    trace = bool(int(os.environ.get("KERNEL_TRACE", "0")))
    if trace:
        _install_trace_shim()

    nc = _get_program(apply_gb)

    ho_v = H_orig.reshape(BATCH, ORIG, 128, 512)
    hd_v = H_down.reshape(BATCH, DOWN, 128, 256)
    wo = np.ascontiguousarray((0.5 * W_o2n.T).reshape(2, 128, 256))
    wn = np.ascontiguousarray((0.5 * W_n2o.T).reshape(2, 128, 256))
    ident = np.eye(128, dtype=np.float32)

    in_maps = []
    for core in range(NCORES):
        b, h = divmod(core, 2)
        im = {
            "ho": ho_v[b, RPC * h: RPC * (h + 1)],
            "hd": hd_v[b, DRC * h: DRC * (h + 1)],
            "wo": wo, "wn": wn, "ident": ident,
        }
        if apply_gb:
            im["go"] = np.ascontiguousarray(
                np.broadcast_to(g_o2n, (128, 256)))
            im["bo"] = np.ascontiguousarray(
                np.broadcast_to(b_o2n, (128, 256)))
            im["gn"] = np.ascontiguousarray(
                np.broadcast_to(g_n2o, (128, 256)))
            im["bn"] = np.ascontiguousarray(
                np.broadcast_to(b_n2o, (128, 256)))
        in_maps.append(im)

    res = run_bass_kernel_spmd(nc, in_maps, core_ids=list(range(NCORES)),
                               trace=trace)
    LAST_EXEC_NS = res.exec_time_ns
    LAST_RESULTS = res

    H_up = np.empty((BATCH, ORIG * ORIG, F), dtype=np.float32)
    H_new = np.empty((BATCH, DOWN * DOWN, F), dtype=np.float32)
    hu_v = H_up.reshape(BATCH, 2, RPC, 128, 512)
    hn_v = H_new.reshape(BATCH, 2, DRC, 128, 256)
    for core in range(NCORES):
        b, h = divmod(core, 2)
        hu_v[b, h] = res.results[core]["hu"]
        hn_v[b, h] = res.results[core]["hn"]
    return H_up, H_new


# revision 14
# speedup vs baseline: 1.2225x; 1.0148x over previous
"""Trainium2 Bass kernel for CrossLayerFeatureUpdate (2x2 block-pooled GNN
message passing between a 256x256 grid and its 128x128 downsample).

Math notes (from the reference adjacency construction):
  - orig=256, down=128, f=orig//down=2, so the 4 neighbors of down node
    (di,dj) are exactly the non-overlapping 2x2 block {2di,2di+1}x{2dj,2dj+1}.
    Every orig node has degree exactly 1 => all adjacency vals == 0.5.
  - down path:  H_new = relu(LN((0.5 * pool2x2sum(H_orig)) @ W_o2n.T))
  - up path:    rows of (0.5 * upsample(H_down)) @ W_n2o.T are identical for
    the 4 children of one down node, and LN/relu act rowwise, so we compute
    y = relu(LN((0.5 * H_down) @ W_n2o.T)) once per down node and write it
    to the 4 child positions.
  - The 0.5 is folded into the weight matrices on the host.

Layout trick: one orig grid-row r (256 cols x 256 feats, contiguous) viewed
as [128, 512] puts col 2p features in free [0:256] and col 2p+1 features in
free [256:512] of partition p. So 2x2 pooling becomes plain aligned adds of
[128,256] slices of two contiguous row loads, and the up-path duplication is
one [128,512] tile = [y | y] written to two consecutive rows.

Sharding: 8 cores = batch(4) x row-half(2). Per core: 32MB+8MB in,
32MB+8MB out, all contiguous DMA. No cross-core communication.
"""

import os
import numpy as np

BATCH, ORIG, DOWN, F = 4, 256, 128, 256
EPS = 1e-5
NCORES = 8
RPC = ORIG // 2   # orig grid-rows per core (128)
DRC = DOWN // 2   # down grid-rows per core (64)

LAST_EXEC_NS = None
LAST_RESULTS = None

_compiled_cache = {}


def _make_tile_context_cls(tile_mod):
    import concourse.bass  # noqa: F401
    from concourse import mybir
    from concourse.vector_clock import ScopedClock

    # opcodes executed by DMA queues (waits ride in DGE descriptors, which
    # have no small per-instruction wait-slot limit)
    _DMA_INSTS = tuple(
        getattr(mybir, n)
        for n in ("InstDMACopy", "InstDMATranspose", "InstTensorLoad",
                  "InstTensorSave")
        if hasattr(mybir, n)
    )

    class TileContextSplitDrain(tile_mod.TileContext):
        """TileContext patched for this walrus build, which rejects >1-2
        sync-wait commands on a single engine instruction: excess waits are
        hoisted onto single-wait EventSemaphore instructions emitted just
        before the instruction on the same engine."""

        MAX_INST_WAITS = 1
        MAX_DMA_WAITS = 1

        def __init__(self, *a, **k):
            super().__init__(*a, **k)
            self._dma_helpers = {}  # engine -> [sem, count]

        def _emit_wait_es(self, engine, wait, update=None):
            es = mybir.InstEventSemaphore(
                name=self.nc.get_next_instruction_name(), ins=[], outs=[])
            es.engine = engine
            es.sync_info = mybir.SyncInfo(
                on_wait=[wait] if wait is not None else [],
                on_update=[update] if update is not None else [])
            super()._add_instruction(es)

        def _add_instruction(self, inst):
            si = inst.sync_info
            if (
                si is not None
                and si.on_wait
                and inst.engine != mybir.EngineType.Unassigned
            ):
                waits = list(si.on_wait)
                if isinstance(inst, _DMA_INSTS):
                    if len(waits) > self.MAX_DMA_WAITS:
                        # DMA descriptors execute on DGE queues, not the
                        # issuing engine stream, so an in-stream wait alone
                        # would not gate them. Hoist every wait onto the
                        # issuing engine; the last hoisted wait bumps a
                        # per-engine helper semaphore that becomes the DMA's
                        # single wait.
                        h = self._dma_helpers.get(inst.engine)
                        if h is None:
                            sem = self.nc.alloc_semaphore(
                                f"dma_wait_helper_{inst.engine.value}")
                            h = self._dma_helpers[inst.engine] = [sem, 0]
                        for w in waits[:-1]:
                            self._emit_wait_es(inst.engine, w)
                        h[1] += 1
                        upd = mybir.SyncUpdate(
                            ant_name=h[0].name, id=h[0].num,
                            sync_type="semaphore",
                            update_mode="sem-inc", update_value=1)
                        self._emit_wait_es(inst.engine, waits[-1], upd)
                        helper_wait = mybir.SyncWait(
                            ant_name=h[0].name, id=h[0].num,
                            sync_type="semaphore",
                            wait_mode="sem-ge-imm", wait_value=h[1])
                        inst.sync_info = mybir.SyncInfo(
                            on_wait=[helper_wait],
                            on_update=list(si.on_update or []))
                elif len(waits) > self.MAX_INST_WAITS:
                    keep = waits[-self.MAX_INST_WAITS:]
                    for w in waits[:-self.MAX_INST_WAITS]:
                        self._emit_wait_es(inst.engine, w)
                    inst.sync_info = mybir.SyncInfo(
                        on_wait=keep, on_update=list(si.on_update or []))
            super()._add_instruction(inst)

        def _drain_and_barrier(self, tick_clock, wait_clock):
            nc = self.nc
            scratch = nc.sync.nop()
            wait_clock.add_sem_waits(
                scratch.ins, ScopedClock({None: tick_clock.global_clock})
            )
            si = scratch.ins.sync_info
            waits = list(si.on_wait) if (si and si.on_wait) else []
            scratch.ins.sync_info = None
            assert self.sems is not None
            num2sem = {h.num: h for h in self.sems.allocated().values()}
            for w in waits:
                nc.sync.wait_ge(num2sem[w.id], w.wait_value)
            nc.sync.drain()
            nc.all_engine_barrier()
            popped = nc._tile_sem_poison_stack.pop()
            assert popped is self._sem_poison
            sems = list(self.sems.allocated().values())
            sems.extend(h[0] for h in self._dma_helpers.values())
            nc.clear_and_free_semaphores(sems)
            nc.all_engine_barrier()

    return TileContextSplitDrain


def _build_program(apply_gb):
    import concourse.bass as bass
    import concourse.tile as tile
    from concourse import mybir

    f32 = mybir.dt.float32
    f32r = mybir.dt.float32r
    ACT = mybir.ActivationFunctionType
    ALU = mybir.AluOpType

    nc = bass.Bass("TRN2", target_bir_lowering=False, debug=False,
                   num_devices=NCORES)

    ho_in = nc.dram_tensor("ho", [RPC, 128, 512], f32, kind="ExternalInput")
    hd_in = nc.dram_tensor("hd", [DRC, 128, 256], f32, kind="ExternalInput")
    wo_in = nc.dram_tensor("wo", [2, 128, 256], f32, kind="ExternalInput")
    wn_in = nc.dram_tensor("wn", [2, 128, 256], f32, kind="ExternalInput")
    id_in = nc.dram_tensor("ident", [128, 128], f32, kind="ExternalInput")
    if apply_gb:
        gb_in = {
            n: nc.dram_tensor(n, [128, 256], f32, kind="ExternalInput")
            for n in ("go", "bo", "gn", "bn")
        }
    hn_out = nc.dram_tensor("hn", [DRC, 128, 256], f32, kind="ExternalOutput")
    hu_out = nc.dram_tensor("hu", [RPC, 128, 512], f32, kind="ExternalOutput")

    TC = _make_tile_context_cls(tile)

    with TC(nc) as tc:
        import contextlib
        with contextlib.ExitStack() as ctx:
            consts = ctx.enter_context(tc.tile_pool(name="consts", bufs=1))
            rin = ctx.enter_context(tc.tile_pool(name="rin", bufs=4))
            hdin = ctx.enter_context(tc.tile_pool(name="hdin", bufs=4))
            dpool = ctx.enter_context(tc.tile_pool(name="dmt", bufs=4))
            outp = ctx.enter_context(tc.tile_pool(name="outp", bufs=4))
            stats = ctx.enter_context(tc.tile_pool(name="stats", bufs=12))
            tpool = ctx.enter_context(
                tc.tile_pool(name="tp", bufs=3, space="PSUM"))
            opool = ctx.enter_context(
                tc.tile_pool(name="op", bufs=3, space="PSUM"))

            w_o = consts.tile([128, 2, 256], f32r)
            w_n = consts.tile([128, 2, 256], f32r)
            ident = consts.tile([128, 128], f32)
            eps_t = consts.tile([128, 1], f32)
            nc.vector.memset(eps_t[:], EPS)
            for c in (0, 1):
                nc.gpsimd.dma_start(out=w_o[:, c, :], in_=wo_in[c])
                nc.gpsimd.dma_start(out=w_n[:, c, :], in_=wn_in[c])
            nc.sync.dma_start(out=ident[:], in_=id_in[:])
            if apply_gb:
                gb_sb = {}
                for n in ("go", "bo", "gn", "bn"):
                    t = consts.tile([128, 256], f32, tag=f"gb_{n}")
                    nc.sync.dma_start(out=t[:], in_=gb_in[n][:])
                    gb_sb[n] = t

            def layernorm_relu(psum_t, out_ap, g_tile, b_tile):
                """out = relu(LN(psum_t) * g + b); writes [128,256] to out_ap."""
                st = stats.tile([128, 6], f32, tag="st")
                nc.vector.bn_stats(out=st[:], in_=psum_t[:])
                mv = stats.tile([128, 2], f32, tag="mv")
                nc.vector.bn_aggr(out=mv[:], in_=st[:])
                sd = stats.tile([128, 1], f32, tag="sd")
                nc.scalar.activation(out=sd[:], in_=mv[:, 1:2], func=ACT.Sqrt,
                                     bias=eps_t[:], scale=1.0)
                rstd = stats.tile([128, 1], f32, tag="rstd")
                nc.vector.reciprocal(out=rstd[:], in_=sd[:])
                nmr = stats.tile([128, 1], f32, tag="nmr")
                nc.vector.scalar_tensor_tensor(
                    out=nmr[:], in0=mv[:, 0:1], scalar=-1.0, in1=rstd[:],
                    op0=ALU.mult, op1=ALU.mult)
                if g_tile is None:
                    nc.scalar.activation(out=out_ap, in_=psum_t[:],
                                         func=ACT.Relu, scale=rstd[:],
                                         bias=nmr[:])
                else:
                    xn = stats.tile([128, 256], f32, tag="xn")
                    nc.scalar.activation(out=xn[:], in_=psum_t[:],
                                         func=ACT.Identity, scale=rstd[:],
                                         bias=nmr[:])
                    nc.vector.tensor_mul(out=xn[:], in0=xn[:], in1=g_tile[:])
                    nc.vector.tensor_add(out=xn[:], in0=xn[:], in1=b_tile[:])
                    nc.vector.tensor_scalar_max(out=out_ap, in0=xn[:],
                                                scalar1=0.0)

            for di in range(DRC):
                # ---------- down path: pool 2x2 -> matmul -> LN -> relu ----
                rp = rin.tile([128, 1024], f32, tag="rp")
                nc.sync.dma_start(
                    out=rp[:].rearrange("p (r q) -> p r q", r=2),
                    in_=ho_in[2 * di: 2 * di + 2].rearrange("r p q -> p r q"))

                tps = tpool.tile([128, 256], f32, tag="tp")
                for c in (0, 1):
                    for gi, base in enumerate((0, 256, 512, 768)):
                        nc.tensor.matmul(
                            out=tps[:, c * 128:(c + 1) * 128],
                            lhsT=rp[:, base + c * 128: base + c * 128 + 128],
                            rhs=ident[:],
                            is_transpose=True,
                            start=(gi == 0), stop=(gi == 3),
                        )
                dmT = dpool.tile([128, 256], f32r, tag="dmT")
                nc.scalar.copy(out=dmT[:], in_=tps[:])

                ops = opool.tile([128, 256], f32, tag="op")
                for c in (0, 1):
                    nc.tensor.matmul(
                        out=ops[:],
                        lhsT=dmT[:, c * 128:(c + 1) * 128],
                        rhs=w_o[:, c, :],
                        start=(c == 0), stop=(c == 1),
                    )

                hnt = outp.tile([128, 256], f32, tag="hnt")
                layernorm_relu(ops, hnt[:],
                               gb_sb["go"] if apply_gb else None,
                               gb_sb["bo"] if apply_gb else None)
                nc.gpsimd.dma_start(out=hn_out[di], in_=hnt[:])

                # ---------- up path: matmul -> LN -> relu -> duplicate -----
                hdt = hdin.tile([128, 256], f32, tag="hdt")
                nc.gpsimd.dma_start(out=hdt[:], in_=hd_in[di])

                tps2 = tpool.tile([128, 256], f32, tag="tp")
                for c in (0, 1):
                    nc.tensor.matmul(
                        out=tps2[:, c * 128:(c + 1) * 128],
                        lhsT=hdt[:, c * 128:(c + 1) * 128],
                        rhs=ident[:],
                        is_transpose=True,
                        start=True, stop=True,
                    )
                dmT2 = dpool.tile([128, 256], f32r, tag="dmT")
                nc.scalar.copy(out=dmT2[:], in_=tps2[:])

                ops2 = opool.tile([128, 256], f32, tag="op")
                for c in (0, 1):
                    nc.tensor.matmul(
                        out=ops2[:],
                        lhsT=dmT2[:, c * 128:(c + 1) * 128],
                        rhs=w_n[:, c, :],
                        start=(c == 0), stop=(c == 1),
                    )

                y2 = outp.tile([128, 512], f32, tag="y2")
                layernorm_relu(ops2, y2[:, 0:256],
                               gb_sb["gn"] if apply_gb else None,
                               gb_sb["bn"] if apply_gb else None)
                nc.vector.tensor_copy(out=y2[:, 256:512], in_=y2[:, 0:256])
                y2ap = y2[:]
                y2rep = bass.AP(tensor=y2ap.tensor, offset=y2ap.offset,
                                ap=[list(y2ap.ap[0]), [0, 2],
                                    list(y2ap.ap[1])])
                nc.gpsimd.dma_start(
                    out=hu_out[2 * di: 2 * di + 2].rearrange(
                        "r p q -> p r q"),
                    in_=y2rep)

    return nc


def _get_program(apply_gb):
    key = bool(apply_gb)
    if key not in _compiled_cache:
        _compiled_cache[key] = _build_program(key)
    return _compiled_cache[key]


def _install_trace_shim():
    """Register the NTFF profile hook so trace=True works (timing)."""
    import sys, types
    if "antenv.axon_hooks" in sys.modules:
        return
    try:
        from trn_agent_boot.trn_boot import _ntff_profile_via_ctypes
        hook = _ntff_profile_via_ctypes('/opt/axon/libaxon_pjrt.so')
    except Exception:
        return
    mod = types.ModuleType("antenv.axon_hooks")
    mod.get_axon_ntff_profile_hook = lambda: hook
    mod.set_axon_ntff_profile_hook = lambda h: None
    sys.modules["antenv.axon_hooks"] = mod
    import concourse.bass_utils as bu
    bu.upload_artifacts = lambda tmpdir: "local://" + str(tmpdir)


def kernel(H_orig, H_down, W_o2n, W_n2o, g_o2n, b_o2n, g_n2o, b_n2o):
    global LAST_EXEC_NS, LAST_RESULTS
    from concourse.bass_utils import run_bass_kernel_spmd

    H_orig = np.ascontiguousarray(np.asarray(H_orig), dtype=np.float32)
    H_down = np.ascontiguousarray(np.asarray(H_down), dtype=np.float32)
    W_o2n = np.asarray(W_o2n, dtype=np.float32)
    W_n2o = np.asarray(W_n2o, dtype=np.float32)
    g_o2n = np.asarray(g_o2n, dtype=np.float32)
    b_o2n = np.asarray(b_o2n, dtype=np.float32)
    g_n2o = np.asarray(g_n2o, dtype=np.float32)
    b_n2o = np.asarray(b_n2o, dtype=np.float32)

    apply_gb = not (
        np.all(g_o2n == 1.0) and np.all(b_o2n == 0.0)
        and np.all(g_n2o == 1.0) and np.all(b_n2o == 0.0)
    )

    trace = bool(int(os.environ.get("KERNEL_TRACE", "0")))
    if trace:
        _install_trace_shim()

    nc = _get_program(apply_gb)

    ho_v = H_orig.reshape(BATCH, ORIG, 128, 512)
    hd_v = H_down.reshape(BATCH, DOWN, 128, 256)
    wo = np.ascontiguousarray((0.5 * W_o2n.T).reshape(2, 128, 256))
    wn = np.ascontiguousarray((0.5 * W_n2o.T).reshape(2, 128, 256))
    ident = np.eye(128, dtype=np.float32)

    in_maps = []
    for core in range(NCORES):
        b, h = divmod(core, 2)
        im = {
            "ho": ho_v[b, RPC * h: RPC * (h + 1)],
            "hd": hd_v[b, DRC * h: DRC * (h + 1)],
            "wo": wo, "wn": wn, "ident": ident,
        }
        if apply_gb:
            im["go"] = np.ascontiguousarray(
                np.broadcast_to(g_o2n, (128, 256)))
            im["bo"] = np.ascontiguousarray(
                np.broadcast_to(b_o2n, (128, 256)))
            im["gn"] = np.ascontiguousarray(
                np.broadcast_to(g_n2o, (128, 256)))
            im["bn"] = np.ascontiguousarray(
                np.broadcast_to(b_n2o, (128, 256)))
        in_maps.append(im)

    res = run_bass_kernel_spmd(nc, in_maps, core_ids=list(range(NCORES)),
                               trace=trace)
    LAST_EXEC_NS = res.exec_time_ns
    LAST_RESULTS = res

    H_up = np.empty((BATCH, ORIG * ORIG, F), dtype=np.float32)
    H_new = np.empty((BATCH, DOWN * DOWN, F), dtype=np.float32)
    hu_v = H_up.reshape(BATCH, 2, RPC, 128, 512)
    hn_v = H_new.reshape(BATCH, 2, DRC, 128, 256)
    for core in range(NCORES):
        b, h = divmod(core, 2)
        hu_v[b, h] = res.results[core]["hu"]
        hn_v[b, h] = res.results[core]["hn"]
    return H_up, H_new


# revision 16
# speedup vs baseline: 1.2456x; 1.0189x over previous
"""Trainium2 Bass kernel for CrossLayerFeatureUpdate (2x2 block-pooled GNN
message passing between a 256x256 grid and its 128x128 downsample).

Math notes (from the reference adjacency construction):
  - orig=256, down=128, f=orig//down=2, so the 4 neighbors of down node
    (di,dj) are exactly the non-overlapping 2x2 block {2di,2di+1}x{2dj,2dj+1}.
    Every orig node has degree exactly 1 => all adjacency vals == 0.5.
  - down path:  H_new = relu(LN((0.5 * pool2x2sum(H_orig)) @ W_o2n.T))
  - up path:    rows of (0.5 * upsample(H_down)) @ W_n2o.T are identical for
    the 4 children of one down node, and LN/relu act rowwise, so we compute
    y = relu(LN((0.5 * H_down) @ W_n2o.T)) once per down node and write it
    to the 4 child positions.
  - The 0.5 is folded into the weight matrices on the host.

Layout trick: one orig grid-row r (256 cols x 256 feats, contiguous) viewed
as [128, 512] puts col 2p features in free [0:256] and col 2p+1 features in
free [256:512] of partition p. So 2x2 pooling becomes plain aligned adds of
[128,256] slices of two contiguous row loads, and the up-path duplication is
one [128,512] tile = [y | y] written to two consecutive rows.

Sharding: 8 cores = batch(4) x row-half(2). Per core: 32MB+8MB in,
32MB+8MB out, all contiguous DMA. No cross-core communication.
"""

import os
import numpy as np

BATCH, ORIG, DOWN, F = 4, 256, 128, 256
EPS = 1e-5
NCORES = 8
RPC = ORIG // 2   # orig grid-rows per core (128)
DRC = DOWN // 2   # down grid-rows per core (64)

LAST_EXEC_NS = None
LAST_RESULTS = None

_compiled_cache = {}


def _make_tile_context_cls(tile_mod):
    import concourse.bass  # noqa: F401
    from concourse import mybir
    from concourse.vector_clock import ScopedClock

    # opcodes executed by DMA queues (waits ride in DGE descriptors, which
    # have no small per-instruction wait-slot limit)
    _DMA_INSTS = tuple(
        getattr(mybir, n)
        for n in ("InstDMACopy", "InstDMATranspose", "InstTensorLoad",
                  "InstTensorSave")
        if hasattr(mybir, n)
    )

    class TileContextSplitDrain(tile_mod.TileContext):
        """TileContext patched for this walrus build, which rejects >1-2
        sync-wait commands on a single engine instruction: excess waits are
        hoisted onto single-wait EventSemaphore instructions emitted just
        before the instruction on the same engine."""

        MAX_INST_WAITS = 1
        MAX_DMA_WAITS = 1

        def __init__(self, *a, **k):
            super().__init__(*a, **k)
            self._dma_helpers = {}  # engine -> [sem, count]

        def _emit_wait_es(self, engine, wait, update=None):
            es = mybir.InstEventSemaphore(
                name=self.nc.get_next_instruction_name(), ins=[], outs=[])
            es.engine = engine
            es.sync_info = mybir.SyncInfo(
                on_wait=[wait] if wait is not None else [],
                on_update=[update] if update is not None else [])
            super()._add_instruction(es)

        def _add_instruction(self, inst):
            si = inst.sync_info
            if (
                si is not None
                and si.on_wait
                and inst.engine != mybir.EngineType.Unassigned
            ):
                waits = list(si.on_wait)
                if isinstance(inst, _DMA_INSTS):
                    if len(waits) > self.MAX_DMA_WAITS:
                        # DMA descriptors execute on DGE queues, not the
                        # issuing engine stream, so an in-stream wait alone
                        # would not gate them. Hoist every wait onto the
                        # issuing engine; the last hoisted wait bumps a
                        # per-engine helper semaphore that becomes the DMA's
                        # single wait.
                        h = self._dma_helpers.get(inst.engine)
                        if h is None:
                            sem = self.nc.alloc_semaphore(
                                f"dma_wait_helper_{inst.engine.value}")
                            h = self._dma_helpers[inst.engine] = [sem, 0]
                        for w in waits[:-1]:
                            self._emit_wait_es(inst.engine, w)
                        h[1] += 1
                        upd = mybir.SyncUpdate(
                            ant_name=h[0].name, id=h[0].num,
                            sync_type="semaphore",
                            update_mode="sem-inc", update_value=1)
                        self._emit_wait_es(inst.engine, waits[-1], upd)
                        helper_wait = mybir.SyncWait(
                            ant_name=h[0].name, id=h[0].num,
                            sync_type="semaphore",
                            wait_mode="sem-ge-imm", wait_value=h[1])
                        inst.sync_info = mybir.SyncInfo(
                            on_wait=[helper_wait],
                            on_update=list(si.on_update or []))
                elif len(waits) > self.MAX_INST_WAITS:
                    keep = waits[-self.MAX_INST_WAITS:]
                    for w in waits[:-self.MAX_INST_WAITS]:
                        self._emit_wait_es(inst.engine, w)
                    inst.sync_info = mybir.SyncInfo(
                        on_wait=keep, on_update=list(si.on_update or []))
            super()._add_instruction(inst)

        def _drain_and_barrier(self, tick_clock, wait_clock):
            nc = self.nc
            scratch = nc.sync.nop()
            wait_clock.add_sem_waits(
                scratch.ins, ScopedClock({None: tick_clock.global_clock})
            )
            si = scratch.ins.sync_info
            waits = list(si.on_wait) if (si and si.on_wait) else []
            scratch.ins.sync_info = None
            assert self.sems is not None
            num2sem = {h.num: h for h in self.sems.allocated().values()}
            for w in waits:
                nc.sync.wait_ge(num2sem[w.id], w.wait_value)
            nc.sync.drain()
            nc.all_engine_barrier()
            popped = nc._tile_sem_poison_stack.pop()
            assert popped is self._sem_poison
            sems = list(self.sems.allocated().values())
            sems.extend(h[0] for h in self._dma_helpers.values())
            nc.clear_and_free_semaphores(sems)
            nc.all_engine_barrier()

    return TileContextSplitDrain


def _build_program(apply_gb):
    import concourse.bass as bass
    import concourse.tile as tile
    from concourse import mybir

    f32 = mybir.dt.float32
    f32r = mybir.dt.float32r
    ACT = mybir.ActivationFunctionType
    ALU = mybir.AluOpType

    nc = bass.Bass("TRN2", target_bir_lowering=False, debug=False,
                   num_devices=NCORES)

    ho_in = nc.dram_tensor("ho", [RPC, 128, 512], f32, kind="ExternalInput")
    hd_in = nc.dram_tensor("hd", [DRC, 128, 256], f32, kind="ExternalInput")
    wo_in = nc.dram_tensor("wo", [2, 128, 256], f32, kind="ExternalInput")
    wn_in = nc.dram_tensor("wn", [2, 128, 256], f32, kind="ExternalInput")
    id_in = nc.dram_tensor("ident", [128, 128], f32, kind="ExternalInput")
    if apply_gb:
        gb_in = {
            n: nc.dram_tensor(n, [128, 256], f32, kind="ExternalInput")
            for n in ("go", "bo", "gn", "bn")
        }
    hn_out = nc.dram_tensor("hn", [DRC, 128, 256], f32, kind="ExternalOutput")
    hu_out = nc.dram_tensor("hu", [RPC, 128, 512], f32, kind="ExternalOutput")

    TC = _make_tile_context_cls(tile)

    with TC(nc) as tc:
        import contextlib
        with contextlib.ExitStack() as ctx:
            consts = ctx.enter_context(tc.tile_pool(name="consts", bufs=1))
            rin = ctx.enter_context(tc.tile_pool(name="rin", bufs=4))
            hdin = ctx.enter_context(tc.tile_pool(name="hdin", bufs=4))
            dpool = ctx.enter_context(tc.tile_pool(name="dmt", bufs=4))
            outp = ctx.enter_context(tc.tile_pool(name="outp", bufs=4))
            stats = ctx.enter_context(tc.tile_pool(name="stats", bufs=12))
            tpool = ctx.enter_context(
                tc.tile_pool(name="tp", bufs=3, space="PSUM"))
            opool = ctx.enter_context(
                tc.tile_pool(name="op", bufs=3, space="PSUM"))

            w_o = consts.tile([128, 2, 256], f32r)
            w_n = consts.tile([128, 2, 256], f32r)
            ident = consts.tile([128, 128], f32)
            eps_t = consts.tile([128, 1], f32)
            nc.vector.memset(eps_t[:], EPS)
            for c in (0, 1):
                nc.gpsimd.dma_start(out=w_o[:, c, :], in_=wo_in[c])
                nc.gpsimd.dma_start(out=w_n[:, c, :], in_=wn_in[c])
            nc.sync.dma_start(out=ident[:], in_=id_in[:])
            if apply_gb:
                gb_sb = {}
                for n in ("go", "bo", "gn", "bn"):
                    t = consts.tile([128, 256], f32, tag=f"gb_{n}")
                    nc.sync.dma_start(out=t[:], in_=gb_in[n][:])
                    gb_sb[n] = t

            def layernorm_relu(psum_t, out_ap, g_tile, b_tile):
                """out = relu(LN(psum_t) * g + b); writes [128,256] to out_ap."""
                st = stats.tile([128, 6], f32, tag="st")
                nc.vector.bn_stats(out=st[:], in_=psum_t[:])
                mv = stats.tile([128, 2], f32, tag="mv")
                nc.vector.bn_aggr(out=mv[:], in_=st[:])
                sd = stats.tile([128, 1], f32, tag="sd")
                nc.scalar.activation(out=sd[:], in_=mv[:, 1:2], func=ACT.Sqrt,
                                     bias=eps_t[:], scale=1.0)
                rstd = stats.tile([128, 1], f32, tag="rstd")
                nc.vector.reciprocal(out=rstd[:], in_=sd[:])
                nmr = stats.tile([128, 1], f32, tag="nmr")
                nc.vector.scalar_tensor_tensor(
                    out=nmr[:], in0=mv[:, 0:1], scalar=-1.0, in1=rstd[:],
                    op0=ALU.mult, op1=ALU.mult)
                if g_tile is None:
                    nc.scalar.activation(out=out_ap, in_=psum_t[:],
                                         func=ACT.Relu, scale=rstd[:],
                                         bias=nmr[:])
                else:
                    xn = stats.tile([128, 256], f32, tag="xn")
                    nc.scalar.activation(out=xn[:], in_=psum_t[:],
                                         func=ACT.Identity, scale=rstd[:],
                                         bias=nmr[:])
                    nc.vector.tensor_mul(out=xn[:], in0=xn[:], in1=g_tile[:])
                    nc.vector.tensor_add(out=xn[:], in0=xn[:], in1=b_tile[:])
                    nc.vector.tensor_scalar_max(out=out_ap, in0=xn[:],
                                                scalar1=0.0)

            BI = 4  # down-rows per DMA batch
            for ti in range(DRC // BI):
                # batched loads: BI down-rows = 2*BI orig rows (1 MB) and
                # BI H_down rows, one DMA instruction each.
                rp = rin.tile([128, 2 * BI, 512], f32, tag="rp")
                nc.sync.dma_start(
                    out=rp[:],
                    in_=ho_in[2 * BI * ti: 2 * BI * (ti + 1)].rearrange(
                        "r p q -> p r q"))
                hdt = hdin.tile([128, BI, 256], f32, tag="hdt")
                nc.gpsimd.dma_start(
                    out=hdt[:],
                    in_=hd_in[BI * ti: BI * (ti + 1)].rearrange(
                        "r p q -> p r q"))

                hnt = outp.tile([128, BI, 256], f32, tag="hnt")
                y2 = outp.tile([128, BI, 512], f32, tag="y2")

                for k in range(BI):
                    # ------- down path: pool 2x2 -> matmul -> LN -> relu ---
                    tps = tpool.tile([128, 256], f32, tag="tp")
                    for c in (0, 1):
                        for gi, (r, half) in enumerate(
                                ((2 * k, 0), (2 * k, 1),
                                 (2 * k + 1, 0), (2 * k + 1, 1))):
                            base = half * 256 + c * 128
                            nc.tensor.matmul(
                                out=tps[:, c * 128:(c + 1) * 128],
                                lhsT=rp[:, r, base: base + 128],
                                rhs=ident[:],
                                is_transpose=True,
                                start=(gi == 0), stop=(gi == 3),
                            )
                    dmT = dpool.tile([128, 256], f32r, tag="dmT")
                    nc.scalar.copy(out=dmT[:], in_=tps[:])

                    ops = opool.tile([128, 256], f32, tag="op")
                    for c in (0, 1):
                        nc.tensor.matmul(
                            out=ops[:],
                            lhsT=dmT[:, c * 128:(c + 1) * 128],
                            rhs=w_o[:, c, :],
                            start=(c == 0), stop=(c == 1),
                        )
                    layernorm_relu(ops, hnt[:, k, :],
                                   gb_sb["go"] if apply_gb else None,
                                   gb_sb["bo"] if apply_gb else None)

                    # ------- up path: matmul -> LN -> relu -> duplicate ----
                    tps2 = tpool.tile([128, 256], f32, tag="tp")
                    for c in (0, 1):
                        nc.tensor.matmul(
                            out=tps2[:, c * 128:(c + 1) * 128],
                            lhsT=hdt[:, k, c * 128:(c + 1) * 128],
                            rhs=ident[:],
                            is_transpose=True,
                            start=True, stop=True,
                        )
                    dmT2 = dpool.tile([128, 256], f32r, tag="dmT")
                    nc.scalar.copy(out=dmT2[:], in_=tps2[:])

                    ops2 = opool.tile([128, 256], f32, tag="op")
                    for c in (0, 1):
                        nc.tensor.matmul(
                            out=ops2[:],
                            lhsT=dmT2[:, c * 128:(c + 1) * 128],
                            rhs=w_n[:, c, :],
                            start=(c == 0), stop=(c == 1),
                        )
                    layernorm_relu(ops2, y2[:, k, 0:256],
                                   gb_sb["gn"] if apply_gb else None,
                                   gb_sb["bn"] if apply_gb else None)
                    nc.vector.tensor_copy(out=y2[:, k, 256:512],
                                          in_=y2[:, k, 0:256])

                # batched stores: one DMA each
                nc.sync.dma_start(
                    out=hn_out[BI * ti: BI * (ti + 1)].rearrange(
                        "r p q -> p r q"),
                    in_=hnt[:])
                hu_blk = hu_out[2 * BI * ti: 2 * BI * (ti + 1)].rearrange(
                    "(k dup) p q -> dup p k q", dup=2)
                nc.gpsimd.dma_start(out=hu_blk[0], in_=y2[:])
                nc.gpsimd.dma_start(out=hu_blk[1], in_=y2[:])

    return nc


def _get_program(apply_gb):
    key = bool(apply_gb)
    if key not in _compiled_cache:
        _compiled_cache[key] = _build_program(key)
    return _compiled_cache[key]


def _install_trace_shim():
    """Register the NTFF profile hook so trace=True works (timing)."""
    import sys, types
    if "antenv.axon_hooks" in sys.modules:
        return
    try:
        from trn_agent_boot.trn_boot import _ntff_profile_via_ctypes
        hook = _ntff_profile_via_ctypes('/opt/axon/libaxon_pjrt.so')
    except Exception:
        return
    mod = types.ModuleType("antenv.axon_hooks")
    mod.get_axon_ntff_profile_hook = lambda: hook
    mod.set_axon_ntff_profile_hook = lambda h: None
    sys.modules["antenv.axon_hooks"] = mod
    import concourse.bass_utils as bu
    bu.upload_artifacts = lambda tmpdir: "local://" + str(tmpdir)


def kernel(H_orig, H_down, W_o2n, W_n2o, g_o2n, b_o2n, g_n2o, b_n2o):
    global LAST_EXEC_NS, LAST_RESULTS
    from concourse.bass_utils import run_bass_kernel_spmd

    H_orig = np.ascontiguousarray(np.asarray(H_orig), dtype=np.float32)
    H_down = np.ascontiguousarray(np.asarray(H_down), dtype=np.float32)
    W_o2n = np.asarray(W_o2n, dtype=np.float32)
    W_n2o = np.asarray(W_n2o, dtype=np.float32)
    g_o2n = np.asarray(g_o2n, dtype=np.float32)
    b_o2n = np.asarray(b_o2n, dtype=np.float32)
    g_n2o = np.asarray(g_n2o, dtype=np.float32)
    b_n2o = np.asarray(b_n2o, dtype=np.float32)

    apply_gb = not (
        np.all(g_o2n == 1.0) and np.all(b_o2n == 0.0)
        and np.all(g_n2o == 1.0) and np.all(b_n2o == 0.0)
    )

    trace = bool(int(os.environ.get("KERNEL_TRACE", "0")))
    if trace:
        _install_trace_shim()

    nc = _get_program(apply_gb)

    ho_v = H_orig.reshape(BATCH, ORIG, 128, 512)
    hd_v = H_down.reshape(BATCH, DOWN, 128, 256)
    wo = np.ascontiguousarray((0.5 * W_o2n.T).reshape(2, 128, 256))
    wn = np.ascontiguousarray((0.5 * W_n2o.T).reshape(2, 128, 256))
    ident = np.eye(128, dtype=np.float32)

    in_maps = []
    for core in range(NCORES):
        b, h = divmod(core, 2)
        im = {
            "ho": ho_v[b, RPC * h: RPC * (h + 1)],
            "hd": hd_v[b, DRC * h: DRC * (h + 1)],
            "wo": wo, "wn": wn, "ident": ident,
        }
        if apply_gb:
            im["go"] = np.ascontiguousarray(
                np.broadcast_to(g_o2n, (128, 256)))
            im["bo"] = np.ascontiguousarray(
                np.broadcast_to(b_o2n, (128, 256)))
            im["gn"] = np.ascontiguousarray(
                np.broadcast_to(g_n2o, (128, 256)))
            im["bn"] = np.ascontiguousarray(
                np.broadcast_to(b_n2o, (128, 256)))
        in_maps.append(im)

    res = run_bass_kernel_spmd(nc, in_maps, core_ids=list(range(NCORES)),
                               trace=trace)
    LAST_EXEC_NS = res.exec_time_ns
    LAST_RESULTS = res

    H_up = np.empty((BATCH, ORIG * ORIG, F), dtype=np.float32)
    H_new = np.empty((BATCH, DOWN * DOWN, F), dtype=np.float32)
    hu_v = H_up.reshape(BATCH, 2, RPC, 128, 512)
    hn_v = H_new.reshape(BATCH, 2, DRC, 128, 256)
    for core in range(NCORES):
        b, h = divmod(core, 2)
        hu_v[b, h] = res.results[core]["hu"]
        hn_v[b, h] = res.results[core]["hn"]
    return H_up, H_new


# revision 17
# speedup vs baseline: 1.2553x; 1.0078x over previous
"""Trainium2 Bass kernel for CrossLayerFeatureUpdate (2x2 block-pooled GNN
message passing between a 256x256 grid and its 128x128 downsample).

Math notes (from the reference adjacency construction):
  - orig=256, down=128, f=orig//down=2, so the 4 neighbors of down node
    (di,dj) are exactly the non-overlapping 2x2 block {2di,2di+1}x{2dj,2dj+1}.
    Every orig node has degree exactly 1 => all adjacency vals == 0.5.
  - down path:  H_new = relu(LN((0.5 * pool2x2sum(H_orig)) @ W_o2n.T))
  - up path:    rows of (0.5 * upsample(H_down)) @ W_n2o.T are identical for
    the 4 children of one down node, and LN/relu act rowwise, so we compute
    y = relu(LN((0.5 * H_down) @ W_n2o.T)) once per down node and write it
    to the 4 child positions.
  - The 0.5 is folded into the weight matrices on the host.

Layout trick: one orig grid-row r (256 cols x 256 feats, contiguous) viewed
as [128, 512] puts col 2p features in free [0:256] and col 2p+1 features in
free [256:512] of partition p. So 2x2 pooling becomes plain aligned adds of
[128,256] slices of two contiguous row loads, and the up-path duplication is
one [128,512] tile = [y | y] written to two consecutive rows.

Sharding: 8 cores = batch(4) x row-half(2). Per core: 32MB+8MB in,
32MB+8MB out, all contiguous DMA. No cross-core communication.
"""

import os
import numpy as np

BATCH, ORIG, DOWN, F = 4, 256, 128, 256
EPS = 1e-5
NCORES = 8
RPC = ORIG // 2   # orig grid-rows per core (128)
DRC = DOWN // 2   # down grid-rows per core (64)

LAST_EXEC_NS = None
LAST_RESULTS = None

_compiled_cache = {}


def _make_tile_context_cls(tile_mod):
    import concourse.bass  # noqa: F401
    from concourse import mybir
    from concourse.vector_clock import ScopedClock

    # opcodes executed by DMA queues (waits ride in DGE descriptors, which
    # have no small per-instruction wait-slot limit)
    _DMA_INSTS = tuple(
        getattr(mybir, n)
        for n in ("InstDMACopy", "InstDMATranspose", "InstTensorLoad",
                  "InstTensorSave")
        if hasattr(mybir, n)
    )

    class TileContextSplitDrain(tile_mod.TileContext):
        """TileContext patched for this walrus build, which rejects >1-2
        sync-wait commands on a single engine instruction: excess waits are
        hoisted onto single-wait EventSemaphore instructions emitted just
        before the instruction on the same engine."""

        MAX_INST_WAITS = 1
        MAX_DMA_WAITS = 1

        def __init__(self, *a, **k):
            super().__init__(*a, **k)
            self._dma_helpers = {}  # engine -> [sem, count]

        def _emit_wait_es(self, engine, wait, update=None):
            es = mybir.InstEventSemaphore(
                name=self.nc.get_next_instruction_name(), ins=[], outs=[])
            es.engine = engine
            es.sync_info = mybir.SyncInfo(
                on_wait=[wait] if wait is not None else [],
                on_update=[update] if update is not None else [])
            super()._add_instruction(es)

        def _add_instruction(self, inst):
            si = inst.sync_info
            if (
                si is not None
                and si.on_wait
                and inst.engine != mybir.EngineType.Unassigned
            ):
                waits = list(si.on_wait)
                if isinstance(inst, _DMA_INSTS):
                    if len(waits) > self.MAX_DMA_WAITS:
                        # DMA descriptors execute on DGE queues, not the
                        # issuing engine stream, so an in-stream wait alone
                        # would not gate them. Hoist every wait onto the
                        # issuing engine; the last hoisted wait bumps a
                        # per-engine helper semaphore that becomes the DMA's
                        # single wait.
                        h = self._dma_helpers.get(inst.engine)
                        if h is None:
                            sem = self.nc.alloc_semaphore(
                                f"dma_wait_helper_{inst.engine.value}")
                            h = self._dma_helpers[inst.engine] = [sem, 0]
                        for w in waits[:-1]:
                            self._emit_wait_es(inst.engine, w)
                        h[1] += 1
                        upd = mybir.SyncUpdate(
                            ant_name=h[0].name, id=h[0].num,
                            sync_type="semaphore",
                            update_mode="sem-inc", update_value=1)
                        self._emit_wait_es(inst.engine, waits[-1], upd)
                        helper_wait = mybir.SyncWait(
                            ant_name=h[0].name, id=h[0].num,
                            sync_type="semaphore",
                            wait_mode="sem-ge-imm", wait_value=h[1])
                        inst.sync_info = mybir.SyncInfo(
                            on_wait=[helper_wait],
                            on_update=list(si.on_update or []))
                elif len(waits) > self.MAX_INST_WAITS:
                    keep = waits[-self.MAX_INST_WAITS:]
                    for w in waits[:-self.MAX_INST_WAITS]:
                        self._emit_wait_es(inst.engine, w)
                    inst.sync_info = mybir.SyncInfo(
                        on_wait=keep, on_update=list(si.on_update or []))
            super()._add_instruction(inst)

        def _drain_and_barrier(self, tick_clock, wait_clock):
            nc = self.nc
            scratch = nc.sync.nop()
            wait_clock.add_sem_waits(
                scratch.ins, ScopedClock({None: tick_clock.global_clock})
            )
            si = scratch.ins.sync_info
            waits = list(si.on_wait) if (si and si.on_wait) else []
            scratch.ins.sync_info = None
            assert self.sems is not None
            num2sem = {h.num: h for h in self.sems.allocated().values()}
            for w in waits:
                nc.sync.wait_ge(num2sem[w.id], w.wait_value)
            nc.sync.drain()
            nc.all_engine_barrier()
            popped = nc._tile_sem_poison_stack.pop()
            assert popped is self._sem_poison
            sems = list(self.sems.allocated().values())
            sems.extend(h[0] for h in self._dma_helpers.values())
            nc.clear_and_free_semaphores(sems)
            nc.all_engine_barrier()

    return TileContextSplitDrain


def _build_program(apply_gb):
    import concourse.bass as bass
    import concourse.tile as tile
    from concourse import mybir

    f32 = mybir.dt.float32
    f32r = mybir.dt.float32r
    ACT = mybir.ActivationFunctionType
    ALU = mybir.AluOpType

    nc = bass.Bass("TRN2", target_bir_lowering=False, debug=False,
                   num_devices=NCORES)

    ho_in = nc.dram_tensor("ho", [RPC, 128, 512], f32, kind="ExternalInput")
    hd_in = nc.dram_tensor("hd", [DRC, 128, 256], f32, kind="ExternalInput")
    wo_in = nc.dram_tensor("wo", [2, 128, 256], f32, kind="ExternalInput")
    wn_in = nc.dram_tensor("wn", [2, 128, 256], f32, kind="ExternalInput")
    id_in = nc.dram_tensor("ident", [128, 128], f32, kind="ExternalInput")
    if apply_gb:
        gb_in = {
            n: nc.dram_tensor(n, [128, 256], f32, kind="ExternalInput")
            for n in ("go", "bo", "gn", "bn")
        }
    hn_out = nc.dram_tensor("hn", [DRC, 128, 256], f32, kind="ExternalOutput")
    hu_out = nc.dram_tensor("hu", [RPC, 128, 512], f32, kind="ExternalOutput")

    TC = _make_tile_context_cls(tile)

    with TC(nc) as tc:
        import contextlib
        with contextlib.ExitStack() as ctx:
            consts = ctx.enter_context(tc.tile_pool(name="consts", bufs=1))
            rin = ctx.enter_context(tc.tile_pool(name="rin", bufs=4))
            hdin = ctx.enter_context(tc.tile_pool(name="hdin", bufs=4))
            dpool = ctx.enter_context(tc.tile_pool(name="dmt", bufs=4))
            outp = ctx.enter_context(tc.tile_pool(name="outp", bufs=4))
            stats = ctx.enter_context(tc.tile_pool(name="stats", bufs=12))
            tpool = ctx.enter_context(
                tc.tile_pool(name="tp", bufs=3, space="PSUM"))
            opool = ctx.enter_context(
                tc.tile_pool(name="op", bufs=3, space="PSUM"))

            w_o = consts.tile([128, 2, 256], f32r)
            w_n = consts.tile([128, 2, 256], f32r)
            ident = consts.tile([128, 128], f32)
            eps_t = consts.tile([128, 1], f32)
            nc.vector.memset(eps_t[:], EPS)
            for c in (0, 1):
                nc.gpsimd.dma_start(out=w_o[:, c, :], in_=wo_in[c])
                nc.gpsimd.dma_start(out=w_n[:, c, :], in_=wn_in[c])
            nc.sync.dma_start(out=ident[:], in_=id_in[:])
            if apply_gb:
                gb_sb = {}
                for n in ("go", "bo", "gn", "bn"):
                    t = consts.tile([128, 256], f32, tag=f"gb_{n}")
                    nc.sync.dma_start(out=t[:], in_=gb_in[n][:])
                    gb_sb[n] = t

            def layernorm_relu(psum_t, out_ap, g_tile, b_tile):
                """out = relu(LN(psum_t) * g + b); writes [128,256] to out_ap."""
                st = stats.tile([128, 6], f32, tag="st")
                nc.vector.bn_stats(out=st[:], in_=psum_t[:])
                mv = stats.tile([128, 2], f32, tag="mv")
                nc.vector.bn_aggr(out=mv[:], in_=st[:])
                sd = stats.tile([128, 1], f32, tag="sd")
                nc.scalar.activation(out=sd[:], in_=mv[:, 1:2], func=ACT.Sqrt,
                                     bias=eps_t[:], scale=1.0)
                rstd = stats.tile([128, 1], f32, tag="rstd")
                nc.vector.reciprocal(out=rstd[:], in_=sd[:])
                nmr = stats.tile([128, 1], f32, tag="nmr")
                nc.vector.scalar_tensor_tensor(
                    out=nmr[:], in0=mv[:, 0:1], scalar=-1.0, in1=rstd[:],
                    op0=ALU.mult, op1=ALU.mult)
                if g_tile is None:
                    nc.scalar.activation(out=out_ap, in_=psum_t[:],
                                         func=ACT.Relu, scale=rstd[:],
                                         bias=nmr[:])
                else:
                    xn = stats.tile([128, 256], f32, tag="xn")
                    nc.scalar.activation(out=xn[:], in_=psum_t[:],
                                         func=ACT.Identity, scale=rstd[:],
                                         bias=nmr[:])
                    nc.vector.tensor_mul(out=xn[:], in0=xn[:], in1=g_tile[:])
                    nc.vector.tensor_add(out=xn[:], in0=xn[:], in1=b_tile[:])
                    nc.vector.tensor_scalar_max(out=out_ap, in0=xn[:],
                                                scalar1=0.0)

            BI = 4  # down-rows per DMA batch
            NTI = DRC // BI

            hd_tiles = {}

            def load_hd(ti):
                t = hdin.tile([128, BI, 256], f32, tag="hdt")
                nc.scalar.dma_start(
                    out=t[:],
                    in_=hd_in[BI * ti: BI * (ti + 1)].rearrange(
                        "r p q -> p r q"))
                hd_tiles[ti] = t

            load_hd(0)
            for ti in range(NTI):
                # batched loads: BI down-rows = 2*BI orig rows (1 MB) and
                # BI H_down rows, one DMA instruction each.
                rp = rin.tile([128, 2 * BI, 512], f32, tag="rp")
                nc.sync.dma_start(
                    out=rp[:],
                    in_=ho_in[2 * BI * ti: 2 * BI * (ti + 1)].rearrange(
                        "r p q -> p r q"))
                if ti + 1 < NTI:
                    load_hd(ti + 1)
                hdt = hd_tiles.pop(ti)

                hnt = outp.tile([128, BI, 256], f32, tag="hnt")
                y2 = outp.tile([128, BI, 512], f32, tag="y2")

                for k in range(BI):
                    # ------- down path: pool 2x2 -> matmul -> LN -> relu ---
                    tps = tpool.tile([128, 256], f32, tag="tp")
                    for c in (0, 1):
                        for gi, (r, half) in enumerate(
                                ((2 * k, 0), (2 * k, 1),
                                 (2 * k + 1, 0), (2 * k + 1, 1))):
                            base = half * 256 + c * 128
                            nc.tensor.matmul(
                                out=tps[:, c * 128:(c + 1) * 128],
                                lhsT=rp[:, r, base: base + 128],
                                rhs=ident[:],
                                is_transpose=True,
                                start=(gi == 0), stop=(gi == 3),
                            )
                    dmT = dpool.tile([128, 256], f32r, tag="dmT")
                    nc.vector.tensor_copy(out=dmT[:], in_=tps[:])

                    ops = opool.tile([128, 256], f32, tag="op")
                    for c in (0, 1):
                        nc.tensor.matmul(
                            out=ops[:],
                            lhsT=dmT[:, c * 128:(c + 1) * 128],
                            rhs=w_o[:, c, :],
                            start=(c == 0), stop=(c == 1),
                        )
                    layernorm_relu(ops, hnt[:, k, :],
                                   gb_sb["go"] if apply_gb else None,
                                   gb_sb["bo"] if apply_gb else None)

                    # ------- up path: matmul -> LN -> relu -> duplicate ----
                    tps2 = tpool.tile([128, 256], f32, tag="tp")
                    for c in (0, 1):
                        nc.tensor.matmul(
                            out=tps2[:, c * 128:(c + 1) * 128],
                            lhsT=hdt[:, k, c * 128:(c + 1) * 128],
                            rhs=ident[:],
                            is_transpose=True,
                            start=True, stop=True,
                        )
                    dmT2 = dpool.tile([128, 256], f32r, tag="dmT")
                    nc.vector.tensor_copy(out=dmT2[:], in_=tps2[:])

                    ops2 = opool.tile([128, 256], f32, tag="op")
                    for c in (0, 1):
                        nc.tensor.matmul(
                            out=ops2[:],
                            lhsT=dmT2[:, c * 128:(c + 1) * 128],
                            rhs=w_n[:, c, :],
                            start=(c == 0), stop=(c == 1),
                        )
                    layernorm_relu(ops2, y2[:, k, 0:256],
                                   gb_sb["gn"] if apply_gb else None,
                                   gb_sb["bn"] if apply_gb else None)
                    nc.vector.tensor_copy(out=y2[:, k, 256:512],
                                          in_=y2[:, k, 0:256])

                # batched stores: one DMA each
                nc.scalar.dma_start(
                    out=hn_out[BI * ti: BI * (ti + 1)].rearrange(
                        "r p q -> p r q"),
                    in_=hnt[:])
                hu_blk = hu_out[2 * BI * ti: 2 * BI * (ti + 1)].rearrange(
                    "(k dup) p q -> dup p k q", dup=2)
                nc.gpsimd.dma_start(out=hu_blk[0], in_=y2[:])
                nc.gpsimd.dma_start(out=hu_blk[1], in_=y2[:])

    return nc


def _get_program(apply_gb):
    key = bool(apply_gb)
    if key not in _compiled_cache:
        _compiled_cache[key] = _build_program(key)
    return _compiled_cache[key]


def _install_trace_shim():
    """Register the NTFF profile hook so trace=True works (timing)."""
    import sys, types
    if "antenv.axon_hooks" in sys.modules:
        return
    try:
        from trn_agent_boot.trn_boot import _ntff_profile_via_ctypes
        hook = _ntff_profile_via_ctypes('/opt/axon/libaxon_pjrt.so')
    except Exception:
        return
    mod = types.ModuleType("antenv.axon_hooks")
    mod.get_axon_ntff_profile_hook = lambda: hook
    mod.set_axon_ntff_profile_hook = lambda h: None
    sys.modules["antenv.axon_hooks"] = mod
    import concourse.bass_utils as bu
    bu.upload_artifacts = lambda tmpdir: "local://" + str(tmpdir)


def kernel(H_orig, H_down, W_o2n, W_n2o, g_o2n, b_o2n, g_n2o, b_n2o):
    global LAST_EXEC_NS, LAST_RESULTS
    from concourse.bass_utils import run_bass_kernel_spmd

    H_orig = np.ascontiguousarray(np.asarray(H_orig), dtype=np.float32)
    H_down = np.ascontiguousarray(np.asarray(H_down), dtype=np.float32)
    W_o2n = np.asarray(W_o2n, dtype=np.float32)
    W_n2o = np.asarray(W_n2o, dtype=np.float32)
    g_o2n = np.asarray(g_o2n, dtype=np.float32)
    b_o2n = np.asarray(b_o2n, dtype=np.float32)
    g_n2o = np.asarray(g_n2o, dtype=np.float32)
    b_n2o = np.asarray(b_n2o, dtype=np.float32)

    apply_gb = not (
        np.all(g_o2n == 1.0) and np.all(b_o2n == 0.0)
        and np.all(g_n2o == 1.0) and np.all(b_n2o == 0.0)
    )

    trace = bool(int(os.environ.get("KERNEL_TRACE", "0")))
    if trace:
        _install_trace_shim()

    nc = _get_program(apply_gb)

    ho_v = H_orig.reshape(BATCH, ORIG, 128, 512)
    hd_v = H_down.reshape(BATCH, DOWN, 128, 256)
    wo = np.ascontiguousarray((0.5 * W_o2n.T).reshape(2, 128, 256))
    wn = np.ascontiguousarray((0.5 * W_n2o.T).reshape(2, 128, 256))
    ident = np.eye(128, dtype=np.float32)

    in_maps = []
    for core in range(NCORES):
        b, h = divmod(core, 2)
        im = {
            "ho": ho_v[b, RPC * h: RPC * (h + 1)],
            "hd": hd_v[b, DRC * h: DRC * (h + 1)],
            "wo": wo, "wn": wn, "ident": ident,
        }
        if apply_gb:
            im["go"] = np.ascontiguousarray(
                np.broadcast_to(g_o2n, (128, 256)))
            im["bo"] = np.ascontiguousarray(
                np.broadcast_to(b_o2n, (128, 256)))
            im["gn"] = np.ascontiguousarray(
                np.broadcast_to(g_n2o, (128, 256)))
            im["bn"] = np.ascontiguousarray(
                np.broadcast_to(b_n2o, (128, 256)))
        in_maps.append(im)

    res = run_bass_kernel_spmd(nc, in_maps, core_ids=list(range(NCORES)),
                               trace=trace)
    LAST_EXEC_NS = res.exec_time_ns
    LAST_RESULTS = res

    H_up = np.empty((BATCH, ORIG * ORIG, F), dtype=np.float32)
    H_new = np.empty((BATCH, DOWN * DOWN, F), dtype=np.float32)
    hu_v = H_up.reshape(BATCH, 2, RPC, 128, 512)
    hn_v = H_new.reshape(BATCH, 2, DRC, 128, 256)
    for core in range(NCORES):
        b, h = divmod(core, 2)
        hu_v[b, h] = res.results[core]["hu"]
        hn_v[b, h] = res.results[core]["hn"]
    return H_up, H_new
